# revision 10
# baseline (speedup 1.0000x reference)
"""Trainium2 Bass kernel for fp8 quantize-dequantize DenseGeneral + gelu.

Computes: out = gelu(qdq_e4m3fn(x) @ qdq_e4m3fn(W) + round_bf16(bias))
with delayed-scaling fp8 quantization (scale = amax/448 over full tensor,
folded with the amax history), reproducing reference.py numerics.

Distribution (8 NeuronCores, tensor-parallel on F):
  - x [8192, 4096] is token-sharded for the quantize phase (1024 tokens/core);
    each core computes a local abs-max, quantizes+transposes its shard to
    fp8, then an AllGather replicates the full quantized x^T to all cores.
  - W [4096, 16384] and bias are column-sharded (2048 cols/core). Local
    abs-max; quantized W shard stays SBUF-resident.
  - A single 2-float AllReduce(max) produces the global amaxes.
  - Each core computes out[:, its 2048 cols] and the host concatenates.

fp8 trick: TRN's float8e4 has max 240 (OCP e4m3fn has 448). We store q/2:
multiplying by a power of two preserves round-to-nearest decisions on the
3-bit-mantissa grid, so RNE(v/2) in TRN-fp8 == RNE(v)/2 in e4m3fn for all
|v| >= 2^-5 (below that, absolute error <= 2^-9 * scale - negligible).
The factor 4 is folded into the output scale C = 4 * s_x * s_w.
The matmul runs in fp8 DoubleRow mode (2 fp8 MACs/cell/cycle).
"""

import sys

sys.path.insert(0, "/opt/trn_rl_repo")

import numpy as np
from contextlib import ExitStack

import concourse.bass as bass
import concourse.mybir as mybir
import concourse.tile as tile
from concourse import bacc, bass_isa
from concourse.bass_utils import run_bass_kernel_spmd
from concourse.bass_interp import get_hw_module
from concourse.masks import make_identity

F32 = mybir.dt.float32
BF16 = mybir.dt.bfloat16
FP8 = mybir.dt.float8e4
AX = mybir.AxisListType
ALU = mybir.AluOpType
ACTF = mybir.ActivationFunctionType
DR = mybir.MatmulPerfMode.DoubleRow

# Problem shapes (hardcoded per contract)
B, S, D, F = 4, 2048, 4096, 16384
T = B * S
NCORES = 8
HL = 16
E4M3_MAX = 448.0


def build_program(t_shard, d, f_shard, n_cores, hl=HL, act_fn=ACTF.Gelu_apprx_tanh):
    """Build the SPMD per-core bass program. Same program on every core;
    per-core behavior differs only through the input shards."""
    P = 128
    t_total = t_shard * n_cores
    t_tiles = t_shard // P          # token tiles in this core's x shard
    d_tiles = d // P                # contraction subtiles
    NF = 512                        # psum free dim
    n_tiles = f_shard // NF
    d_half = min(d, 2048)           # x amax/quant DMA chunk (free dim)
    d_chunks = d // d_half
    MSUP = min(512, t_shard)        # tokens per streamed lhsT tile
    assert d_tiles % 2 == 0, "DoubleRow needs an even number of k-subtiles"

    nc = bacc.Bacc(
        "TRN2",
        target_bir_lowering=False,
        debug=False,
        num_devices=n_cores,
    )

    x_sh = nc.dram_tensor("x_shard", [t_shard, d], F32, kind="ExternalInput")
    w_sh = nc.dram_tensor("w_shard", [d, f_shard], F32, kind="ExternalInput")
    b_sh = nc.dram_tensor("bias_shard", [1, f_shard], F32, kind="ExternalInput")
    ih = nc.dram_tensor("in_hist", [1, hl], F32, kind="ExternalInput")
    kh = nc.dram_tensor("k_hist", [1, hl], F32, kind="ExternalInput")
    out_sh = nc.dram_tensor("out_shard", [t_total, f_shard], F32, kind="ExternalOutput")

    rg = [list(range(n_cores))]
    shared = "Shared" if n_cores > 4 else "Local"

    with tile.TileContext(nc) as tc, ExitStack() as ctx:
        const = ctx.enter_context(tc.tile_pool(name="const", bufs=1))
        small = ctx.enter_context(tc.tile_pool(name="small", bufs=1))
        xs = ctx.enter_context(tc.tile_pool(name="xs", bufs=3))
        ws = ctx.enter_context(tc.tile_pool(name="ws", bufs=2))
        qwp = ctx.enter_context(tc.tile_pool(name="qw", bufs=1))
        qxs = ctx.enter_context(tc.tile_pool(name="qxs", bufs=2))
        lhsp = ctx.enter_context(tc.tile_pool(name="lhs", bufs=2))
        stg = ctx.enter_context(tc.tile_pool(name="stg", bufs=3))
        psum = ctx.enter_context(tc.tile_pool(name="psum", bufs=8, space="PSUM"))
        dram = ctx.enter_context(tc.tile_pool(name="dram", bufs=1, space="DRAM"))

        # ---- constants ----
        ident = const.tile([P, P], F32)
        make_identity(nc, ident)
        zbias = const.tile([P, 1], F32)
        nc.gpsimd.memset(zbias[:], 0.0)

        # ---- phase 1: local abs-max of x shard and w shard ----
        histx = small.tile([1, hl], F32)
        nc.sync.dma_start(histx[:], ih[:])
        histw = small.tile([1, hl], F32)
        nc.sync.dma_start(histw[:], kh[:])

        xacc = small.tile([P, t_tiles * d_chunks], F32)
        for i in range(t_tiles):
            for h in range(d_chunks):
                xt = xs.tile([P, d_half], F32)
                nc.sync.dma_start(
                    xt[:], x_sh[i * P : (i + 1) * P, h * d_half : (h + 1) * d_half]
                )
                c = i * d_chunks + h
                nc.vector.reduce_max(
                    xacc[:, c : c + 1], xt[:], axis=AX.X, apply_absolute_value=True
                )
        wacc = small.tile([P, d_tiles], F32)
        for j in range(d_tiles):
            wt = ws.tile([P, f_shard], F32)
            nc.sync.dma_start(wt[:], w_sh[j * P : (j + 1) * P, :])
            nc.vector.reduce_max(
                wacc[:, j : j + 1], wt[:], axis=AX.X, apply_absolute_value=True
            )

        xmax_p = small.tile([P, 1], F32)
        nc.vector.reduce_max(xmax_p[:], xacc[:], axis=AX.X)
        wmax_p = small.tile([P, 1], F32)
        nc.vector.reduce_max(wmax_p[:], wacc[:], axis=AX.X)
        xmax_a = small.tile([P, 1], F32)
        nc.gpsimd.partition_all_reduce(xmax_a[:], xmax_p[:], 128, bass_isa.ReduceOp.max)
        wmax_a = small.tile([P, 1], F32)
        nc.gpsimd.partition_all_reduce(wmax_a[:], wmax_p[:], 128, bass_isa.ReduceOp.max)

        pair = small.tile([1, 2], F32)
        nc.vector.tensor_copy(pair[:, 0:1], xmax_a[0:1, :])
        nc.vector.tensor_copy(pair[:, 1:2], wmax_a[0:1, :])

        ar_in = dram.tile([1, 2], F32)
        ar_out = dram.tile([1, 2], F32, addr_space=shared)
        nc.sync.dma_start(ar_in[:], pair[:])
        nc.gpsimd.collective_compute(
            "AllReduce",
            ALU.max,
            replica_groups=rg,
            ins=[ar_in[:].opt()],
            outs=[ar_out[:].opt()],
        )
        gmax = small.tile([1, 2], F32)
        nc.sync.dma_start(gmax[:], ar_out[:])

        # ---- phase 2: scales ----
        # reference: hist' = [amax_now, hist[0:HL-1]]; amax = max(hist')
        #            sf = 448/amax ; s = 1/sf (dequant scale)
        # ours:      r_half = 0.5*sf (quant multiplier, half-scale trick)
        #            C = 4 * s_x * s_w (output scale)
        def scales(gm, hist, sfx):
            hmx = small.tile([1, 1], F32, name=f"hmx_{sfx}")
            nc.vector.reduce_max(hmx[:], hist[:, 0 : hl - 1], axis=AX.X)
            amax = small.tile([1, 1], F32, name=f"amax_{sfx}")
            nc.vector.tensor_tensor(amax[:], gm, hmx[:], op=ALU.max)
            ra = small.tile([1, 1], F32, name=f"ra_{sfx}")
            nc.vector.reciprocal(ra[:], amax[:])
            sf = small.tile([1, 1], F32, name=f"sf_{sfx}")
            nc.vector.tensor_scalar_mul(sf[:], ra[:], E4M3_MAX)
            s = small.tile([1, 1], F32, name=f"s_{sfx}")
            nc.vector.reciprocal(s[:], sf[:])
            rh = small.tile([1, 1], F32, name=f"rh_{sfx}")
            nc.vector.tensor_scalar_mul(rh[:], sf[:], 0.5)
            return s, rh

        s_x, rh_x = scales(gmax[:, 0:1], histx, "x")
        s_w, rh_w = scales(gmax[:, 1:2], histw, "w")

        Cs = small.tile([1, 1], F32)
        nc.vector.tensor_tensor(Cs[:], s_x[:], s_w[:], op=ALU.mult)
        nc.vector.tensor_scalar_mul(Cs[:], Cs[:], 4.0)
        rC = small.tile([1, 1], F32)
        nc.vector.reciprocal(rC[:], Cs[:])

        rhx_b = small.tile([P, 1], F32)
        nc.gpsimd.partition_broadcast(rhx_b[:], rh_x[:])
        rhw_b = small.tile([P, 1], F32)
        nc.gpsimd.partition_broadcast(rhw_b[:], rh_w[:])
        C_b = small.tile([P, 1], F32)
        nc.gpsimd.partition_broadcast(C_b[:], Cs[:])

        # bias: fp32 -> bf16 -> fp32, then pre-divide by C, broadcast to 128 parts
        btmp = small.tile([1, f_shard], F32)
        nc.sync.dma_start(btmp[:], b_sh[:])
        bbf = small.tile([1, f_shard], BF16)
        nc.vector.tensor_copy(bbf[:], btmp[:])
        nc.vector.tensor_copy(btmp[:], bbf[:])
        nc.vector.tensor_scalar_mul(btmp[:], btmp[:], rC[:])
        bP = small.tile([P, f_shard], F32)
        nc.gpsimd.partition_broadcast(bP[:], btmp[:])

        # ---- phase 3: quantize + transpose x shard -> qxT (fp8, [d, t_shard]) ----
        qxt_dram = dram.tile([d, t_shard], FP8)
        for i in range(t_tiles):
            qstage = qxs.tile([P, d_tiles, P], FP8)  # [p, s, m] = qxT[s*128+p, i*128+m]
            for h in range(d_chunks):
                xt = xs.tile([P, d_half], F32)
                nc.sync.dma_start(
                    xt[:], x_sh[i * P : (i + 1) * P, h * d_half : (h + 1) * d_half]
                )
                for j in range(d_half // P):
                    dj = h * (d_half // P) + j
                    pt = psum.tile([P, NF], F32, tag="ps")
                    nc.tensor.transpose(
                        pt[:, 0:P], xt[:, j * P : (j + 1) * P], ident[:]
                    )
                    nc.vector.tensor_scalar(
                        out=qstage[:, dj, :],
                        in0=pt[:, 0:P],
                        scalar1=rhx_b[:],
                        scalar2=None,
                        op0=ALU.mult,
                    )
            # one DMA: SBUF [p, s, m] -> DRAM rows s*128+p, cols i*128+m
            qv = qxt_dram[:].rearrange("(s p) m -> p s m", p=P, s=d_tiles)
            nc.sync.dma_start(qv[:, :, i * P : (i + 1) * P], qstage[:])

        qxt_all = dram.tile([n_cores * d, t_shard], FP8, addr_space=shared)
        nc.gpsimd.collective_compute(
            "AllGather",
            ALU.bypass,
            replica_groups=rg,
            ins=[qxt_dram[:].opt()],
            outs=[qxt_all[:].opt()],
        )

        # ---- phase 4: quantize w shard (SBUF resident, [p, s, f]) ----
        qw_sb = qwp.tile([P, d_tiles, f_shard], FP8)
        for j in range(d_tiles):
            wt = ws.tile([P, f_shard], F32)
            nc.sync.dma_start(wt[:], w_sh[j * P : (j + 1) * P, :])
            nc.vector.tensor_scalar(
                out=qw_sb[:, j, :],
                in0=wt[:],
                scalar1=rhw_b[:],
                scalar2=None,
                op0=ALU.mult,
            )

        # ---- phase 5: matmul + epilogue ----
        # out[tok, f] = gelu(C * (sum_d qxT[d, tok] * qw[d, f] + bias/C))
        qall = qxt_all[:].rearrange(
            "(b s p) m -> b p s m", b=n_cores, s=d_tiles, p=P
        )
        for blk in range(n_cores):
            for half in range(t_shard // MSUP):
                lh = lhsp.tile([P, d_tiles, MSUP], FP8)
                nc.sync.dma_start(
                    lh[:], qall[blk][:, :, half * MSUP : (half + 1) * MSUP]
                )
                for mt in range(MSUP // P):
                    pss = [
                        psum.tile([P, NF], F32, tag="ps", name=f"mmps{n}")
                        for n in range(n_tiles)
                    ]
                    for ks in range(d_tiles // 2):
                        for n in range(n_tiles):
                            nc.tensor.matmul(
                                pss[n][:],
                                lhsT=lh[:, 2 * ks : 2 * ks + 2, mt * P : (mt + 1) * P],
                                rhs=qw_sb[:, 2 * ks : 2 * ks + 2, n * NF : (n + 1) * NF],
                                start=(ks == 0),
                                stop=(ks == d_tiles // 2 - 1),
                                perf_mode=DR,
                            )
                    row0 = blk * t_shard + half * MSUP + mt * P
                    for n in range(n_tiles):
                        t1 = stg.tile([P, NF], F32)
                        nc.vector.tensor_tensor(
                            t1[:], pss[n][:], bP[:, n * NF : (n + 1) * NF], op=ALU.add
                        )
                        ot = stg.tile([P, NF], F32)
                        nc.scalar.activation(
                            ot[:],
                            t1[:],
                            act_fn,
                            bias=zbias[:],
                            scale=C_b[:],
                        )
                        nc.gpsimd.dma_start(
                            out_sh[row0 : row0 + P, n * NF : (n + 1) * NF], ot[:]
                        )

    nc.compile()
    return nc


_CACHE = {}


def _get_program(t_shard=T // NCORES, d=D, f_shard=F // NCORES, n_cores=NCORES):
    key = (t_shard, d, f_shard, n_cores)
    if key not in _CACHE:
        _CACHE[key] = build_program(t_shard, d, f_shard, n_cores)
    return _CACHE[key]


def make_in_maps(x, w, bias, in_hist, k_hist, n_cores=NCORES):
    t_shard = x.shape[0] // n_cores
    f_shard = w.shape[1] // n_cores
    in_maps = []
    for r in range(n_cores):
        in_maps.append(
            {
                "x_shard": np.ascontiguousarray(
                    x[r * t_shard : (r + 1) * t_shard], dtype=np.float32
                ),
                "w_shard": np.ascontiguousarray(
                    w[:, r * f_shard : (r + 1) * f_shard], dtype=np.float32
                ),
                "bias_shard": np.ascontiguousarray(
                    bias[r * f_shard : (r + 1) * f_shard], dtype=np.float32
                ).reshape(1, f_shard),
                "in_hist": np.asarray(in_hist, np.float32).reshape(1, HL),
                "k_hist": np.asarray(k_hist, np.float32).reshape(1, HL),
            }
        )
    return in_maps


def _install_ntff_shim():
    """Provide antenv.axon_hooks (absent in this image) so bass_utils can
    NTFF-profile under axon, wiring it to libaxon_pjrt's nrt profile API."""
    import sys as _sys
    import types

    if "antenv.axon_hooks" in _sys.modules:
        return
    mod = types.ModuleType("antenv.axon_hooks")
    _state = {"hook": None}
    mod.set_axon_ntff_profile_hook = lambda h: _state.__setitem__("hook", h)
    mod.get_axon_ntff_profile_hook = lambda: _state["hook"]
    _sys.modules["antenv.axon_hooks"] = mod
    import antenv

    antenv.axon_hooks = mod
    try:
        from trn_agent_boot.trn_boot import _ntff_profile_via_ctypes

        mod.set_axon_ntff_profile_hook(
            _ntff_profile_via_ctypes("/opt/axon/libaxon_pjrt.so")
        )
    except Exception as e:
        print(f"ntff shim: hook unavailable ({e}); tracing will degrade")


def run(inputs_dict, trace=False, **kw):
    """Run on the 8 NeuronCores; returns (full_output, BassKernelResults)."""
    if trace:
        _install_ntff_shim()
    x = np.asarray(inputs_dict["inputs"], np.float32).reshape(T, D)
    w = np.asarray(inputs_dict["kernel"], np.float32)
    bias = np.asarray(inputs_dict["bias"], np.float32)
    nc = _get_program()
    in_maps = make_in_maps(
        x, w, bias, inputs_dict["input_amax_history"], inputs_dict["kernel_amax_history"]
    )
    old_m = nc.m
    nc.m = get_hw_module(nc.m)
    try:
        res = run_bass_kernel_spmd(
            nc, in_maps, core_ids=list(range(NCORES)), trace=trace, **kw
        )
    finally:
        nc.m = old_m
    f_shard = F // NCORES
    out = np.concatenate(
        [res.results[r]["out_shard"] for r in range(NCORES)], axis=1
    )
    return out.reshape(B, S, F).astype(np.float32), res


def kernel(**inputs):
    out, _ = run(inputs, trace=False)
    return out


# revision 17
# speedup vs baseline: 1.0184x; 1.0184x over previous
"""Trainium2 Bass kernel for fp8 quantize-dequantize DenseGeneral + gelu.

Computes: out = gelu(qdq_e4m3fn(x) @ qdq_e4m3fn(W) + round_bf16(bias))
with delayed-scaling fp8 quantization (scale = amax/448 over full tensor,
folded with the amax history), reproducing reference.py numerics.

Distribution (8 NeuronCores, tensor-parallel on F):
  - x [8192, 4096] is token-sharded for the quantize phase (1024 tokens/core);
    each core computes a local abs-max, quantizes+transposes its shard to
    fp8, then an AllGather replicates the full quantized x^T to all cores.
  - W [4096, 16384] and bias are column-sharded (2048 cols/core). Local
    abs-max; quantized W shard stays SBUF-resident.
  - A single 2-float AllReduce(max) produces the global amaxes.
  - Each core computes out[:, its 2048 cols] and the host concatenates.

fp8 trick: TRN's float8e4 has max 240 (OCP e4m3fn has 448). We store q/2:
multiplying by a power of two preserves round-to-nearest decisions on the
3-bit-mantissa grid, so RNE(v/2) in TRN-fp8 == RNE(v)/2 in e4m3fn for all
|v| >= 2^-5 (below that, absolute error <= 2^-9 * scale - negligible).
The factor 4 is folded into the output scale C = 4 * s_x * s_w.
The matmul runs in fp8 DoubleRow mode (2 fp8 MACs/cell/cycle).
"""

import sys

sys.path.insert(0, "/opt/trn_rl_repo")

import numpy as np
from contextlib import ExitStack

import concourse.bass as bass
import concourse.mybir as mybir
import concourse.tile as tile
from concourse import bacc, bass_isa
from concourse.bass_utils import run_bass_kernel_spmd
from concourse.bass_interp import get_hw_module
from concourse.masks import make_identity

F32 = mybir.dt.float32
BF16 = mybir.dt.bfloat16
FP8 = mybir.dt.float8e4
AX = mybir.AxisListType
ALU = mybir.AluOpType
ACTF = mybir.ActivationFunctionType
DR = mybir.MatmulPerfMode.DoubleRow

# Problem shapes (hardcoded per contract)
B, S, D, F = 4, 2048, 4096, 16384
T = B * S
NCORES = 8
HL = 16
E4M3_MAX = 448.0


def build_program(t_shard, d, f_shard, n_cores, hl=HL, act_fn=ACTF.Gelu_apprx_tanh):
    """Build the SPMD per-core bass program. Same program on every core;
    per-core behavior differs only through the input shards."""
    P = 128
    t_total = t_shard * n_cores
    t_tiles = t_shard // P          # token tiles in this core's x shard
    d_tiles = d // P                # contraction subtiles
    NF = 512                        # psum free dim
    n_tiles = f_shard // NF
    d_half = min(d, 2048)           # x amax/quant DMA chunk (free dim)
    d_chunks = d // d_half
    MSUP = min(256, t_shard)        # tokens per streamed lhsT tile
    assert d_tiles % 2 == 0, "DoubleRow needs an even number of k-subtiles"

    nc = bacc.Bacc(
        "TRN2",
        target_bir_lowering=False,
        debug=False,
        num_devices=n_cores,
    )

    x_sh = nc.dram_tensor("x_shard", [t_shard, d], F32, kind="ExternalInput")
    w_sh = nc.dram_tensor("w_shard", [d, f_shard], F32, kind="ExternalInput")
    b_sh = nc.dram_tensor("bias_shard", [1, f_shard], F32, kind="ExternalInput")
    ih = nc.dram_tensor("in_hist", [1, hl], F32, kind="ExternalInput")
    kh = nc.dram_tensor("k_hist", [1, hl], F32, kind="ExternalInput")
    out_sh = nc.dram_tensor("out_shard", [t_total, f_shard], F32, kind="ExternalOutput")

    rg = [list(range(n_cores))]
    shared = "Shared" if n_cores > 4 else "Local"

    with tile.TileContext(nc) as tc, ExitStack() as ctx:
        const = ctx.enter_context(tc.tile_pool(name="const", bufs=1))
        small = ctx.enter_context(tc.tile_pool(name="small", bufs=1))
        xs = ctx.enter_context(tc.tile_pool(name="xs", bufs=2))
        ws = ctx.enter_context(tc.tile_pool(name="ws", bufs=3))
        qwp = ctx.enter_context(tc.tile_pool(name="qw", bufs=1))
        qxs = ctx.enter_context(tc.tile_pool(name="qxs", bufs=1))
        lhsp = ctx.enter_context(tc.tile_pool(name="lhs", bufs=2))
        stg = ctx.enter_context(tc.tile_pool(name="stg", bufs=3))
        psum = ctx.enter_context(tc.tile_pool(name="psum", bufs=8, space="PSUM"))
        dram = ctx.enter_context(tc.tile_pool(name="dram", bufs=1, space="DRAM"))

        # ---- constants ----
        ident = const.tile([P, P], F32)
        make_identity(nc, ident)
        zbias = const.tile([P, 1], F32)
        nc.gpsimd.memset(zbias[:], 0.0)

        # ---- phase 1: local abs-max of x shard and w shard ----
        histx = small.tile([1, hl], F32)
        nc.sync.dma_start(histx[:], ih[:])
        histw = small.tile([1, hl], F32)
        nc.sync.dma_start(histw[:], kh[:])

        # interleave x and w streams so DMA + DVE overlap across both
        n_xc = t_tiles * d_chunks
        xacc = small.tile([P, n_xc], F32)
        wacc = small.tile([P, d_tiles], F32)
        for step in range(max(d_tiles, n_xc)):
            if step < n_xc:
                i, h = divmod(step, d_chunks)
                xt = xs.tile([P, d_half], F32)
                nc.sync.dma_start(
                    xt[:], x_sh[i * P : (i + 1) * P, h * d_half : (h + 1) * d_half]
                )
                nc.vector.reduce_max(
                    xacc[:, step : step + 1], xt[:], axis=AX.X,
                    apply_absolute_value=True,
                )
            if step < d_tiles:
                wt = ws.tile([P, f_shard], F32)
                nc.sync.dma_start(wt[:], w_sh[step * P : (step + 1) * P, :])
                nc.vector.reduce_max(
                    wacc[:, step : step + 1], wt[:], axis=AX.X,
                    apply_absolute_value=True,
                )

        def armax(acc, sfx):
            mp = small.tile([P, 1], F32, name=f"mp_{sfx}")
            nc.vector.reduce_max(mp[:], acc[:], axis=AX.X)
            ma = small.tile([P, 1], F32, name=f"ma_{sfx}")
            nc.gpsimd.partition_all_reduce(ma[:], mp[:], 128, bass_isa.ReduceOp.max)
            ar_in = dram.tile([1, 1], F32, name=f"arin_{sfx}")
            ar_out = dram.tile([1, 1], F32, addr_space=shared, name=f"arout_{sfx}")
            nc.sync.dma_start(ar_in[:], ma[0:1, :])
            nc.gpsimd.collective_compute(
                "AllReduce",
                ALU.max,
                replica_groups=rg,
                ins=[ar_in[:].opt()],
                outs=[ar_out[:].opt()],
            )
            g = small.tile([1, 1], F32, name=f"g_{sfx}")
            nc.sync.dma_start(g[:], ar_out[:])
            return g

        # x first: its AR unblocks quantize+AllGather earlier; w's AR only
        # gates the w-quant stream that the matmul k-loop chases.
        gmax_x = armax(xacc, "x")
        gmax_w = armax(wacc, "w")

        # ---- phase 2: scales ----
        # reference: hist' = [amax_now, hist[0:HL-1]]; amax = max(hist')
        #            sf = 448/amax ; s = 1/sf (dequant scale)
        # ours:      r_half = 0.5*sf (quant multiplier, half-scale trick)
        #            C = 4 * s_x * s_w (output scale)
        def scales(gm, hist, sfx):
            hmx = small.tile([1, 1], F32, name=f"hmx_{sfx}")
            nc.vector.reduce_max(hmx[:], hist[:, 0 : hl - 1], axis=AX.X)
            amax = small.tile([1, 1], F32, name=f"amax_{sfx}")
            nc.vector.tensor_tensor(amax[:], gm, hmx[:], op=ALU.max)
            ra = small.tile([1, 1], F32, name=f"ra_{sfx}")
            nc.vector.reciprocal(ra[:], amax[:])
            sf = small.tile([1, 1], F32, name=f"sf_{sfx}")
            nc.vector.tensor_scalar_mul(sf[:], ra[:], E4M3_MAX)
            s = small.tile([1, 1], F32, name=f"s_{sfx}")
            nc.vector.reciprocal(s[:], sf[:])
            rh = small.tile([1, 1], F32, name=f"rh_{sfx}")
            nc.vector.tensor_scalar_mul(rh[:], sf[:], 0.5)
            return s, rh

        s_x, rh_x = scales(gmax_x[:], histx, "x")
        s_w, rh_w = scales(gmax_w[:], histw, "w")

        Cs = small.tile([1, 1], F32)
        nc.vector.tensor_tensor(Cs[:], s_x[:], s_w[:], op=ALU.mult)
        nc.vector.tensor_scalar_mul(Cs[:], Cs[:], 4.0)
        rC = small.tile([1, 1], F32)
        nc.vector.reciprocal(rC[:], Cs[:])

        rhx_b = small.tile([P, 1], F32)
        nc.gpsimd.partition_broadcast(rhx_b[:], rh_x[:])
        rhw_b = small.tile([P, 1], F32)
        nc.gpsimd.partition_broadcast(rhw_b[:], rh_w[:])
        C_b = small.tile([P, 1], F32)
        nc.gpsimd.partition_broadcast(C_b[:], Cs[:])

        # bias: fp32 -> bf16 -> fp32, then pre-divide by C, broadcast to 128 parts
        btmp = small.tile([1, f_shard], F32)
        nc.sync.dma_start(btmp[:], b_sh[:])
        bbf = small.tile([1, f_shard], BF16)
        nc.vector.tensor_copy(bbf[:], btmp[:])
        nc.vector.tensor_copy(btmp[:], bbf[:])
        nc.vector.tensor_scalar_mul(btmp[:], btmp[:], rC[:])
        bP = small.tile([P, f_shard], F32)
        nc.gpsimd.partition_broadcast(bP[:], btmp[:])

        # ---- phase 3: quantize + transpose x shard -> qxT (fp8, [d, t_shard]) ----
        # full quantized shard stays SBUF-resident so the DRAM writes are
        # 32 fat [128, t_shard] transfers (1KB lines) instead of 128B lines.
        qxt_dram = dram.tile([d, t_shard], FP8)
        qxt_sb = qxs.tile([P, d_tiles, t_shard], FP8)
        for i in range(t_tiles):
            for h in range(d_chunks):
                xt = xs.tile([P, d_half], F32)
                nc.sync.dma_start(
                    xt[:], x_sh[i * P : (i + 1) * P, h * d_half : (h + 1) * d_half]
                )
                for j in range(d_half // P):
                    dj = h * (d_half // P) + j
                    pt = psum.tile([P, NF], F32, tag="ps")
                    nc.tensor.transpose(
                        pt[:, 0:P], xt[:, j * P : (j + 1) * P], ident[:]
                    )
                    nc.vector.tensor_scalar(
                        out=qxt_sb[:, dj, i * P : (i + 1) * P],
                        in0=pt[:, 0:P],
                        scalar1=rhx_b[:],
                        scalar2=None,
                        op0=ALU.mult,
                    )
        for dj in range(d_tiles):
            nc.sync.dma_start(
                qxt_dram[dj * P : (dj + 1) * P, :], qxt_sb[:, dj, :]
            )

        qxt_all = dram.tile([n_cores * d, t_shard], FP8, addr_space=shared)
        nc.gpsimd.collective_compute(
            "AllGather",
            ALU.bypass,
            replica_groups=rg,
            ins=[qxt_dram[:].opt()],
            outs=[qxt_all[:].opt()],
        )

        # ---- phase 4: quantize w shard (SBUF resident) ----
        # one tile per DoubleRow pair so the matmul k-loop can stream behind
        # the quantization instead of waiting for the whole [d, f] tensor.
        qw_tiles = [
            qwp.tile([P, 2, f_shard], FP8, name=f"qwt{s}")
            for s in range(d_tiles // 2)
        ]
        for j in range(d_tiles):
            wt = ws.tile([P, f_shard], F32)
            nc.sync.dma_start(wt[:], w_sh[j * P : (j + 1) * P, :])
            nc.vector.tensor_scalar(
                out=qw_tiles[j // 2][:, j % 2, :],
                in0=wt[:],
                scalar1=rhw_b[:],
                scalar2=None,
                op0=ALU.mult,
            )

        # ---- phase 5: matmul + epilogue ----
        # out[tok, f] = gelu(C * (sum_d qxT[d, tok] * qw[d, f] + bias/C))
        qall = qxt_all[:].rearrange(
            "(b s p) m -> b p s m", b=n_cores, s=d_tiles, p=P
        )
        for blk in range(n_cores):
            for half in range(t_shard // MSUP):
                lh = lhsp.tile([P, d_tiles, MSUP], FP8)
                nc.sync.dma_start(
                    lh[:], qall[blk][:, :, half * MSUP : (half + 1) * MSUP]
                )
                for mt in range(MSUP // P):
                    pss = [
                        psum.tile([P, NF], F32, tag="ps", name=f"mmps{n}")
                        for n in range(n_tiles)
                    ]
                    for ks in range(d_tiles // 2):
                        for n in range(n_tiles):
                            nc.tensor.matmul(
                                pss[n][:],
                                lhsT=lh[:, 2 * ks : 2 * ks + 2, mt * P : (mt + 1) * P],
                                rhs=qw_tiles[ks][:, :, n * NF : (n + 1) * NF],
                                start=(ks == 0),
                                stop=(ks == d_tiles // 2 - 1),
                                perf_mode=DR,
                            )
                    row0 = blk * t_shard + half * MSUP + mt * P
                    for n in range(n_tiles):
                        t1 = stg.tile([P, NF], F32)
                        nc.vector.tensor_tensor(
                            t1[:], pss[n][:], bP[:, n * NF : (n + 1) * NF], op=ALU.add
                        )
                        ot = stg.tile([P, NF], F32)
                        nc.scalar.activation(
                            ot[:],
                            t1[:],
                            act_fn,
                            bias=zbias[:],
                            scale=C_b[:],
                        )
                        nc.gpsimd.dma_start(
                            out_sh[row0 : row0 + P, n * NF : (n + 1) * NF], ot[:]
                        )

    nc.compile()
    return nc


_CACHE = {}


def _get_program(t_shard=T // NCORES, d=D, f_shard=F // NCORES, n_cores=NCORES):
    key = (t_shard, d, f_shard, n_cores)
    if key not in _CACHE:
        _CACHE[key] = build_program(t_shard, d, f_shard, n_cores)
    return _CACHE[key]


def make_in_maps(x, w, bias, in_hist, k_hist, n_cores=NCORES):
    t_shard = x.shape[0] // n_cores
    f_shard = w.shape[1] // n_cores
    in_maps = []
    for r in range(n_cores):
        in_maps.append(
            {
                "x_shard": np.ascontiguousarray(
                    x[r * t_shard : (r + 1) * t_shard], dtype=np.float32
                ),
                "w_shard": np.ascontiguousarray(
                    w[:, r * f_shard : (r + 1) * f_shard], dtype=np.float32
                ),
                "bias_shard": np.ascontiguousarray(
                    bias[r * f_shard : (r + 1) * f_shard], dtype=np.float32
                ).reshape(1, f_shard),
                "in_hist": np.asarray(in_hist, np.float32).reshape(1, HL),
                "k_hist": np.asarray(k_hist, np.float32).reshape(1, HL),
            }
        )
    return in_maps


def _install_ntff_shim():
    """Provide antenv.axon_hooks (absent in this image) so bass_utils can
    NTFF-profile under axon, wiring it to libaxon_pjrt's nrt profile API."""
    import sys as _sys
    import types

    if "antenv.axon_hooks" in _sys.modules:
        return
    mod = types.ModuleType("antenv.axon_hooks")
    _state = {"hook": None}
    mod.set_axon_ntff_profile_hook = lambda h: _state.__setitem__("hook", h)
    mod.get_axon_ntff_profile_hook = lambda: _state["hook"]
    _sys.modules["antenv.axon_hooks"] = mod
    import antenv

    antenv.axon_hooks = mod
    try:
        from trn_agent_boot.trn_boot import _ntff_profile_via_ctypes

        mod.set_axon_ntff_profile_hook(
            _ntff_profile_via_ctypes("/opt/axon/libaxon_pjrt.so")
        )
    except Exception as e:
        print(f"ntff shim: hook unavailable ({e}); tracing will degrade")


def run(inputs_dict, trace=False, **kw):
    """Run on the 8 NeuronCores; returns (full_output, BassKernelResults)."""
    if trace:
        _install_ntff_shim()
    x = np.asarray(inputs_dict["inputs"], np.float32).reshape(T, D)
    w = np.asarray(inputs_dict["kernel"], np.float32)
    bias = np.asarray(inputs_dict["bias"], np.float32)
    nc = _get_program()
    in_maps = make_in_maps(
        x, w, bias, inputs_dict["input_amax_history"], inputs_dict["kernel_amax_history"]
    )
    old_m = nc.m
    nc.m = get_hw_module(nc.m)
    try:
        res = run_bass_kernel_spmd(
            nc, in_maps, core_ids=list(range(NCORES)), trace=trace, **kw
        )
    finally:
        nc.m = old_m
    f_shard = F // NCORES
    out = np.concatenate(
        [res.results[r]["out_shard"] for r in range(NCORES)], axis=1
    )
    return out.reshape(B, S, F).astype(np.float32), res


def kernel(**inputs):
    out, _ = run(inputs, trace=False)
    return out


# revision 20
# speedup vs baseline: 1.0255x; 1.0069x over previous
"""Trainium2 Bass kernel for fp8 quantize-dequantize DenseGeneral + gelu.

Computes: out = gelu(qdq_e4m3fn(x) @ qdq_e4m3fn(W) + round_bf16(bias))
with delayed-scaling fp8 quantization (scale = amax/448 over full tensor,
folded with the amax history), reproducing reference.py numerics.

Distribution (8 NeuronCores, tensor-parallel on F):
  - x [8192, 4096] is token-sharded for the quantize phase (1024 tokens/core);
    each core computes a local abs-max, quantizes+transposes its shard to
    fp8, then an AllGather replicates the full quantized x^T to all cores.
  - W [4096, 16384] and bias are column-sharded (2048 cols/core). Local
    abs-max; quantized W shard stays SBUF-resident.
  - A single 2-float AllReduce(max) produces the global amaxes.
  - Each core computes out[:, its 2048 cols] and the host concatenates.

fp8 trick: TRN's float8e4 has max 240 (OCP e4m3fn has 448). We store q/2:
multiplying by a power of two preserves round-to-nearest decisions on the
3-bit-mantissa grid, so RNE(v/2) in TRN-fp8 == RNE(v)/2 in e4m3fn for all
|v| >= 2^-5 (below that, absolute error <= 2^-9 * scale - negligible).
The factor 4 is folded into the output scale C = 4 * s_x * s_w.
The matmul runs in fp8 DoubleRow mode (2 fp8 MACs/cell/cycle).
"""

import sys

sys.path.insert(0, "/opt/trn_rl_repo")

import numpy as np
from contextlib import ExitStack

import concourse.bass as bass
import concourse.mybir as mybir
import concourse.tile as tile
from concourse import bacc, bass_isa
from concourse.bass_utils import run_bass_kernel_spmd
from concourse.bass_interp import get_hw_module
from concourse.masks import make_identity

F32 = mybir.dt.float32
BF16 = mybir.dt.bfloat16
FP8 = mybir.dt.float8e4
AX = mybir.AxisListType
ALU = mybir.AluOpType
ACTF = mybir.ActivationFunctionType
DR = mybir.MatmulPerfMode.DoubleRow

# Problem shapes (hardcoded per contract)
B, S, D, F = 4, 2048, 4096, 16384
T = B * S
NCORES = 8
HL = 16
E4M3_MAX = 448.0


def build_program(t_shard, d, f_shard, n_cores, hl=HL, act_fn=ACTF.Gelu_apprx_tanh):
    """Build the SPMD per-core bass program. Same program on every core;
    per-core behavior differs only through the input shards."""
    P = 128
    t_total = t_shard * n_cores
    t_tiles = t_shard // P          # token tiles in this core's x shard
    d_tiles = d // P                # contraction subtiles
    NF = 512                        # psum free dim
    n_tiles = f_shard // NF
    d_half = min(d, 2048)           # x amax/quant DMA chunk (free dim)
    d_chunks = d // d_half
    MSUP = min(256, t_shard)        # tokens per streamed lhsT tile
    assert d_tiles % 2 == 0, "DoubleRow needs an even number of k-subtiles"

    nc = bacc.Bacc(
        "TRN2",
        target_bir_lowering=False,
        debug=False,
        num_devices=n_cores,
    )

    x_sh = nc.dram_tensor("x_shard", [t_shard, d], F32, kind="ExternalInput")
    w_sh = nc.dram_tensor("w_shard", [d, f_shard], F32, kind="ExternalInput")
    b_sh = nc.dram_tensor("bias_shard", [1, f_shard], F32, kind="ExternalInput")
    ih = nc.dram_tensor("in_hist", [1, hl], F32, kind="ExternalInput")
    kh = nc.dram_tensor("k_hist", [1, hl], F32, kind="ExternalInput")
    out_sh = nc.dram_tensor("out_shard", [t_total, f_shard], F32, kind="ExternalOutput")

    rg = [list(range(n_cores))]
    shared = "Shared" if n_cores > 4 else "Local"

    with tile.TileContext(nc) as tc, ExitStack() as ctx:
        const = ctx.enter_context(tc.tile_pool(name="const", bufs=1))
        small = ctx.enter_context(tc.tile_pool(name="small", bufs=1))
        xs = ctx.enter_context(tc.tile_pool(name="xs", bufs=2))
        ws = ctx.enter_context(tc.tile_pool(name="ws", bufs=3))
        qwp = ctx.enter_context(tc.tile_pool(name="qw", bufs=1))
        qxs = ctx.enter_context(tc.tile_pool(name="qxs", bufs=1))
        lhsp = ctx.enter_context(tc.tile_pool(name="lhs", bufs=2))
        stg = ctx.enter_context(tc.tile_pool(name="stg", bufs=3))
        psum = ctx.enter_context(tc.tile_pool(name="psum", bufs=8, space="PSUM"))
        dram = ctx.enter_context(tc.tile_pool(name="dram", bufs=1, space="DRAM"))

        # ---- constants ----
        ident = const.tile([P, P], F32)
        make_identity(nc, ident)
        zbias = const.tile([P, 1], F32)
        nc.gpsimd.memset(zbias[:], 0.0)

        # ---- phase 1: local abs-max of x shard and w shard ----
        histx = small.tile([1, hl], F32)
        nc.sync.dma_start(histx[:], ih[:])
        histw = small.tile([1, hl], F32)
        nc.sync.dma_start(histw[:], kh[:])

        # x stream first: its amax -> AR -> quant -> AllGather is the critical
        # chain (the AG costs ~150us on its own); w's chain hides behind it.
        n_xc = t_tiles * d_chunks
        xacc = small.tile([P, n_xc], F32)
        wacc = small.tile([P, d_tiles], F32)
        for step in range(n_xc):
            i, h = divmod(step, d_chunks)
            xt = xs.tile([P, d_half], F32)
            nc.sync.dma_start(
                xt[:], x_sh[i * P : (i + 1) * P, h * d_half : (h + 1) * d_half]
            )
            nc.vector.reduce_max(
                xacc[:, step : step + 1], xt[:], axis=AX.X,
                apply_absolute_value=True,
            )
        for step in range(d_tiles):
            wt = ws.tile([P, f_shard], F32)
            nc.scalar.dma_start(wt[:], w_sh[step * P : (step + 1) * P, :])
            nc.vector.reduce_max(
                wacc[:, step : step + 1], wt[:], axis=AX.X,
                apply_absolute_value=True,
            )

        def armax(acc, sfx):
            mp = small.tile([P, 1], F32, name=f"mp_{sfx}")
            nc.vector.reduce_max(mp[:], acc[:], axis=AX.X)
            ma = small.tile([P, 1], F32, name=f"ma_{sfx}")
            nc.gpsimd.partition_all_reduce(ma[:], mp[:], 128, bass_isa.ReduceOp.max)
            ar_in = dram.tile([1, 1], F32, name=f"arin_{sfx}")
            ar_out = dram.tile([1, 1], F32, addr_space=shared, name=f"arout_{sfx}")
            nc.sync.dma_start(ar_in[:], ma[0:1, :])
            nc.gpsimd.collective_compute(
                "AllReduce",
                ALU.max,
                replica_groups=rg,
                ins=[ar_in[:].opt()],
                outs=[ar_out[:].opt()],
            )
            g = small.tile([1, 1], F32, name=f"g_{sfx}")
            nc.sync.dma_start(g[:], ar_out[:])
            return g

        # x first: its AR unblocks quantize+AllGather earlier; w's AR only
        # gates the w-quant stream that the matmul k-loop chases.
        gmax_x = armax(xacc, "x")
        gmax_w = armax(wacc, "w")

        # ---- phase 2: scales ----
        # reference: hist' = [amax_now, hist[0:HL-1]]; amax = max(hist')
        #            sf = 448/amax ; s = 1/sf (dequant scale)
        # ours:      r_half = 0.5*sf (quant multiplier, half-scale trick)
        #            C = 4 * s_x * s_w (output scale)
        def scales(gm, hist, sfx):
            hmx = small.tile([1, 1], F32, name=f"hmx_{sfx}")
            nc.vector.reduce_max(hmx[:], hist[:, 0 : hl - 1], axis=AX.X)
            amax = small.tile([1, 1], F32, name=f"amax_{sfx}")
            nc.vector.tensor_tensor(amax[:], gm, hmx[:], op=ALU.max)
            ra = small.tile([1, 1], F32, name=f"ra_{sfx}")
            nc.vector.reciprocal(ra[:], amax[:])
            sf = small.tile([1, 1], F32, name=f"sf_{sfx}")
            nc.vector.tensor_scalar_mul(sf[:], ra[:], E4M3_MAX)
            s = small.tile([1, 1], F32, name=f"s_{sfx}")
            nc.vector.reciprocal(s[:], sf[:])
            rh = small.tile([1, 1], F32, name=f"rh_{sfx}")
            nc.vector.tensor_scalar_mul(rh[:], sf[:], 0.5)
            return s, rh

        s_x, rh_x = scales(gmax_x[:], histx, "x")
        s_w, rh_w = scales(gmax_w[:], histw, "w")

        Cs = small.tile([1, 1], F32)
        nc.vector.tensor_tensor(Cs[:], s_x[:], s_w[:], op=ALU.mult)
        nc.vector.tensor_scalar_mul(Cs[:], Cs[:], 4.0)
        rC = small.tile([1, 1], F32)
        nc.vector.reciprocal(rC[:], Cs[:])

        rhx_b = small.tile([P, 1], F32)
        nc.gpsimd.partition_broadcast(rhx_b[:], rh_x[:])
        rhw_b = small.tile([P, 1], F32)
        nc.gpsimd.partition_broadcast(rhw_b[:], rh_w[:])
        C_b = small.tile([P, 1], F32)
        nc.gpsimd.partition_broadcast(C_b[:], Cs[:])

        # bias: fp32 -> bf16 -> fp32, then pre-divide by C, broadcast to 128 parts
        btmp = small.tile([1, f_shard], F32)
        nc.sync.dma_start(btmp[:], b_sh[:])
        bbf = small.tile([1, f_shard], BF16)
        nc.vector.tensor_copy(bbf[:], btmp[:])
        nc.vector.tensor_copy(btmp[:], bbf[:])
        nc.vector.tensor_scalar_mul(btmp[:], btmp[:], rC[:])
        bP = small.tile([P, f_shard], F32)
        nc.gpsimd.partition_broadcast(bP[:], btmp[:])

        # ---- phase 3: quantize + transpose x shard -> qxT (fp8, [d, t_shard]) ----
        # full quantized shard stays SBUF-resident so the DRAM writes are
        # 32 fat [128, t_shard] transfers (1KB lines) instead of 128B lines.
        qxt_dram = dram.tile([d, t_shard], FP8)
        qxt_sb = qxs.tile([P, d_tiles, t_shard], FP8)
        for i in range(t_tiles):
            for h in range(d_chunks):
                xt = xs.tile([P, d_half], F32)
                nc.sync.dma_start(
                    xt[:], x_sh[i * P : (i + 1) * P, h * d_half : (h + 1) * d_half]
                )
                for j in range(d_half // P):
                    dj = h * (d_half // P) + j
                    pt = psum.tile([P, NF], F32, tag="ps")
                    nc.tensor.transpose(
                        pt[:, 0:P], xt[:, j * P : (j + 1) * P], ident[:]
                    )
                    nc.vector.tensor_scalar(
                        out=qxt_sb[:, dj, i * P : (i + 1) * P],
                        in0=pt[:, 0:P],
                        scalar1=rhx_b[:],
                        scalar2=None,
                        op0=ALU.mult,
                    )
        for dj in range(d_tiles):
            nc.sync.dma_start(
                qxt_dram[dj * P : (dj + 1) * P, :], qxt_sb[:, dj, :]
            )

        qxt_all = dram.tile([n_cores * d, t_shard], FP8, addr_space=shared)
        nc.gpsimd.collective_compute(
            "AllGather",
            ALU.bypass,
            replica_groups=rg,
            ins=[qxt_dram[:].opt()],
            outs=[qxt_all[:].opt()],
        )

        # ---- phase 4: quantize w shard (SBUF resident) ----
        # one tile per DoubleRow pair so the matmul k-loop can stream behind
        # the quantization instead of waiting for the whole [d, f] tensor.
        qw_tiles = [
            qwp.tile([P, 2, f_shard], FP8, name=f"qwt{s}")
            for s in range(d_tiles // 2)
        ]
        for j in range(d_tiles):
            wt = ws.tile([P, f_shard], F32)
            nc.sync.dma_start(wt[:], w_sh[j * P : (j + 1) * P, :])
            nc.vector.tensor_scalar(
                out=qw_tiles[j // 2][:, j % 2, :],
                in0=wt[:],
                scalar1=rhw_b[:],
                scalar2=None,
                op0=ALU.mult,
            )

        # ---- phase 5: matmul + epilogue ----
        # out[tok, f] = gelu(C * (sum_d qxT[d, tok] * qw[d, f] + bias/C))
        # Block order is rotated per core: slot 0 is the core's OWN token
        # block, read straight from SBUF (qxt_sb) with no AllGather
        # dependency, so the PE starts while the AG is in flight. Slots 1..7
        # read the AG result at a runtime (partition-id based) offset, and
        # every output DMA row offset is runtime-computed to land the slot at
        # its global token position.
        pid_g = nc.gpsimd.partition_id()
        pid_s = nc.sync.partition_id()
        for slot in range(n_cores):
            bi_g = (pid_g + slot) % n_cores
            row_g = bi_g * t_shard
            if slot != 0:
                bi_s = (pid_s + slot) % n_cores
            for half in range(t_shard // MSUP):
                if slot == 0:
                    lh = qxt_sb
                    coff = half * MSUP
                else:
                    lh = lhsp.tile([P, d_tiles, MSUP], FP8, name="lh")
                    src = qxt_all[bass.ds(bi_s * d, d), :].rearrange(
                        "(s p) m -> p s m", p=P, s=d_tiles
                    )
                    nc.sync.dma_start(
                        lh[:], src[:, :, half * MSUP : (half + 1) * MSUP]
                    )
                    coff = 0
                for mt in range(MSUP // P):
                    pss = [
                        psum.tile([P, NF], F32, tag="ps", name=f"mmps{n}")
                        for n in range(n_tiles)
                    ]
                    mc = coff + mt * P
                    for ks in range(d_tiles // 2):
                        for n in range(n_tiles):
                            nc.tensor.matmul(
                                pss[n][:],
                                lhsT=lh[:, 2 * ks : 2 * ks + 2, mc : mc + P],
                                rhs=qw_tiles[ks][:, :, n * NF : (n + 1) * NF],
                                start=(ks == 0),
                                stop=(ks == d_tiles // 2 - 1),
                                perf_mode=DR,
                            )
                    row0 = row_g + (half * MSUP + mt * P)
                    for n in range(n_tiles):
                        t1 = stg.tile([P, NF], F32)
                        nc.vector.tensor_tensor(
                            t1[:], pss[n][:], bP[:, n * NF : (n + 1) * NF], op=ALU.add
                        )
                        ot = stg.tile([P, NF], F32)
                        nc.scalar.activation(
                            ot[:],
                            t1[:],
                            act_fn,
                            bias=zbias[:],
                            scale=C_b[:],
                        )
                        nc.gpsimd.dma_start(
                            out_sh[bass.ds(row0, P), n * NF : (n + 1) * NF], ot[:]
                        )

    nc.compile()
    return nc


_CACHE = {}


def _get_program(t_shard=T // NCORES, d=D, f_shard=F // NCORES, n_cores=NCORES):
    key = (t_shard, d, f_shard, n_cores)
    if key not in _CACHE:
        _CACHE[key] = build_program(t_shard, d, f_shard, n_cores)
    return _CACHE[key]


def make_in_maps(x, w, bias, in_hist, k_hist, n_cores=NCORES):
    t_shard = x.shape[0] // n_cores
    f_shard = w.shape[1] // n_cores
    in_maps = []
    for r in range(n_cores):
        in_maps.append(
            {
                "x_shard": np.ascontiguousarray(
                    x[r * t_shard : (r + 1) * t_shard], dtype=np.float32
                ),
                "w_shard": np.ascontiguousarray(
                    w[:, r * f_shard : (r + 1) * f_shard], dtype=np.float32
                ),
                "bias_shard": np.ascontiguousarray(
                    bias[r * f_shard : (r + 1) * f_shard], dtype=np.float32
                ).reshape(1, f_shard),
                "in_hist": np.asarray(in_hist, np.float32).reshape(1, HL),
                "k_hist": np.asarray(k_hist, np.float32).reshape(1, HL),
            }
        )
    return in_maps


def _install_ntff_shim():
    """Provide antenv.axon_hooks (absent in this image) so bass_utils can
    NTFF-profile under axon, wiring it to libaxon_pjrt's nrt profile API."""
    import sys as _sys
    import types

    if "antenv.axon_hooks" in _sys.modules:
        return
    mod = types.ModuleType("antenv.axon_hooks")
    _state = {"hook": None}
    mod.set_axon_ntff_profile_hook = lambda h: _state.__setitem__("hook", h)
    mod.get_axon_ntff_profile_hook = lambda: _state["hook"]
    _sys.modules["antenv.axon_hooks"] = mod
    import antenv

    antenv.axon_hooks = mod
    try:
        from trn_agent_boot.trn_boot import _ntff_profile_via_ctypes

        mod.set_axon_ntff_profile_hook(
            _ntff_profile_via_ctypes("/opt/axon/libaxon_pjrt.so")
        )
    except Exception as e:
        print(f"ntff shim: hook unavailable ({e}); tracing will degrade")


def run(inputs_dict, trace=False, **kw):
    """Run on the 8 NeuronCores; returns (full_output, BassKernelResults)."""
    if trace:
        _install_ntff_shim()
    x = np.asarray(inputs_dict["inputs"], np.float32).reshape(T, D)
    w = np.asarray(inputs_dict["kernel"], np.float32)
    bias = np.asarray(inputs_dict["bias"], np.float32)
    nc = _get_program()
    in_maps = make_in_maps(
        x, w, bias, inputs_dict["input_amax_history"], inputs_dict["kernel_amax_history"]
    )
    old_m = nc.m
    nc.m = get_hw_module(nc.m)
    try:
        res = run_bass_kernel_spmd(
            nc, in_maps, core_ids=list(range(NCORES)), trace=trace, **kw
        )
    finally:
        nc.m = old_m
    f_shard = F // NCORES
    out = np.concatenate(
        [res.results[r]["out_shard"] for r in range(NCORES)], axis=1
    )
    return out.reshape(B, S, F).astype(np.float32), res


def kernel(**inputs):
    out, _ = run(inputs, trace=False)
    return out


# revision 25
# speedup vs baseline: 1.0542x; 1.0281x over previous
"""Trainium2 Bass kernel for fp8 quantize-dequantize DenseGeneral + gelu.

Computes: out = gelu(qdq_e4m3fn(x) @ qdq_e4m3fn(W) + round_bf16(bias))
with delayed-scaling fp8 quantization (scale = amax/448 over full tensor,
folded with the amax history), reproducing reference.py numerics.

Distribution (8 NeuronCores, tensor-parallel on F):
  - x [8192, 4096] is token-sharded for the quantize phase (1024 tokens/core);
    each core computes a local abs-max, quantizes+transposes its shard to
    fp8, then an AllGather replicates the full quantized x^T to all cores.
  - W [4096, 16384] and bias are column-sharded (2048 cols/core). Local
    abs-max; quantized W shard stays SBUF-resident.
  - A single 2-float AllReduce(max) produces the global amaxes.
  - Each core computes out[:, its 2048 cols] and the host concatenates.

fp8 trick: TRN's float8e4 has max 240 (OCP e4m3fn has 448). We store q/2:
multiplying by a power of two preserves round-to-nearest decisions on the
3-bit-mantissa grid, so RNE(v/2) in TRN-fp8 == RNE(v)/2 in e4m3fn for all
|v| >= 2^-5 (below that, absolute error <= 2^-9 * scale - negligible).
The factor 4 is folded into the output scale C = 4 * s_x * s_w.
The matmul runs in fp8 DoubleRow mode (2 fp8 MACs/cell/cycle).
"""

import sys

sys.path.insert(0, "/opt/trn_rl_repo")

import numpy as np
from contextlib import ExitStack

import concourse.bass as bass
import concourse.mybir as mybir
import concourse.tile as tile
from concourse import bacc, bass_isa
from concourse.bass_utils import run_bass_kernel_spmd
from concourse.bass_interp import get_hw_module
from concourse.masks import make_identity

F32 = mybir.dt.float32
BF16 = mybir.dt.bfloat16
FP8 = mybir.dt.float8e4
AX = mybir.AxisListType
ALU = mybir.AluOpType
ACTF = mybir.ActivationFunctionType
DR = mybir.MatmulPerfMode.DoubleRow

# Problem shapes (hardcoded per contract)
B, S, D, F = 4, 2048, 4096, 16384
T = B * S
NCORES = 8
HL = 16
E4M3_MAX = 448.0


def build_program(t_shard, d, f_shard, n_cores, hl=HL, act_fn=ACTF.Gelu_apprx_tanh):
    """Build the SPMD per-core bass program. Same program on every core;
    per-core behavior differs only through the input shards."""
    P = 128
    t_total = t_shard * n_cores
    t_tiles = t_shard // P          # token tiles in this core's x shard
    d_tiles = d // P                # contraction subtiles
    NF = 512                        # psum free dim
    n_tiles = f_shard // NF
    d_half = min(d, 2048)           # x amax/quant DMA chunk (free dim)
    d_chunks = d // d_half
    MSUP = min(256, t_shard // 2)   # tokens per streamed lhsT tile
    assert d_tiles % 2 == 0, "DoubleRow needs an even number of k-subtiles"

    nc = bacc.Bacc(
        "TRN2",
        target_bir_lowering=False,
        debug=False,
        num_devices=n_cores,
    )

    x_sh = nc.dram_tensor("x_shard", [t_shard, d], F32, kind="ExternalInput")
    w_sh = nc.dram_tensor("w_shard", [d, f_shard], F32, kind="ExternalInput")
    b_sh = nc.dram_tensor("bias_shard", [1, f_shard], F32, kind="ExternalInput")
    ih = nc.dram_tensor("in_hist", [1, hl], F32, kind="ExternalInput")
    kh = nc.dram_tensor("k_hist", [1, hl], F32, kind="ExternalInput")
    out_sh = nc.dram_tensor("out_shard", [t_total, f_shard], F32, kind="ExternalOutput")

    rg = [list(range(n_cores))]
    shared = "Shared" if n_cores > 4 else "Local"

    with tile.TileContext(nc) as tc, ExitStack() as ctx:
        const = ctx.enter_context(tc.tile_pool(name="const", bufs=1))
        small = ctx.enter_context(tc.tile_pool(name="small", bufs=1))
        xs = ctx.enter_context(tc.tile_pool(name="xs", bufs=2))
        ws = ctx.enter_context(tc.tile_pool(name="ws", bufs=3))
        qwp = ctx.enter_context(tc.tile_pool(name="qw", bufs=1))
        qxs = ctx.enter_context(tc.tile_pool(name="qxs", bufs=1))
        lhsp = ctx.enter_context(tc.tile_pool(name="lhs", bufs=2))
        stg = ctx.enter_context(tc.tile_pool(name="stg", bufs=3))
        psum = ctx.enter_context(tc.tile_pool(name="psum", bufs=8, space="PSUM"))
        dram = ctx.enter_context(tc.tile_pool(name="dram", bufs=1, space="DRAM"))

        # ---- constants ----
        ident = const.tile([P, P], F32)
        make_identity(nc, ident)
        zbias = const.tile([P, 1], F32)
        nc.gpsimd.memset(zbias[:], 0.0)

        # ---- phase 1: local abs-max of x shard and w shard ----
        histx = small.tile([1, hl], F32)
        nc.sync.dma_start(histx[:], ih[:])
        histw = small.tile([1, hl], F32)
        nc.sync.dma_start(histw[:], kh[:])

        # x stream first: its amax -> AR -> quant -> AllGather is the critical
        # chain (the AG costs ~150us on its own); w's chain hides behind it.
        n_xc = t_tiles * d_chunks
        xacc = small.tile([P, n_xc], F32)
        wacc = small.tile([P, d_tiles], F32)
        for step in range(n_xc):
            i, h = divmod(step, d_chunks)
            xt = xs.tile([P, d_half], F32)
            nc.sync.dma_start(
                xt[:], x_sh[i * P : (i + 1) * P, h * d_half : (h + 1) * d_half]
            )
            nc.vector.reduce_max(
                xacc[:, step : step + 1], xt[:], axis=AX.X,
                apply_absolute_value=True,
            )
        def armax(acc, sfx):
            mp = small.tile([P, 1], F32, name=f"mp_{sfx}")
            nc.vector.reduce_max(mp[:], acc[:], axis=AX.X)
            ma = small.tile([P, 1], F32, name=f"ma_{sfx}")
            nc.gpsimd.partition_all_reduce(ma[:], mp[:], 128, bass_isa.ReduceOp.max)
            ar_in = dram.tile([1, 1], F32, name=f"arin_{sfx}")
            ar_out = dram.tile([1, 1], F32, addr_space=shared, name=f"arout_{sfx}")
            nc.sync.dma_start(ar_in[:], ma[0:1, :])
            nc.gpsimd.collective_compute(
                "AllReduce",
                ALU.max,
                replica_groups=rg,
                ins=[ar_in[:].opt()],
                outs=[ar_out[:].opt()],
            )
            g = small.tile([1, 1], F32, name=f"g_{sfx}")
            nc.sync.dma_start(g[:], ar_out[:])
            return g

        # x's AR is emitted before any w work so its final DVE reduce isn't
        # queued behind the 32 w reduces; w's AR only gates the w-quant
        # stream that the matmul k-loop chases.
        gmax_x = armax(xacc, "x")

        for step in range(d_tiles):
            wt = ws.tile([P, f_shard], F32)
            nc.scalar.dma_start(wt[:], w_sh[step * P : (step + 1) * P, :])
            nc.vector.reduce_max(
                wacc[:, step : step + 1], wt[:], axis=AX.X,
                apply_absolute_value=True,
            )
        gmax_w = armax(wacc, "w")

        # ---- phase 2: scales ----
        # reference: hist' = [amax_now, hist[0:HL-1]]; amax = max(hist')
        #            sf = 448/amax ; s = 1/sf (dequant scale)
        # ours:      r_half = 0.5*sf (quant multiplier, half-scale trick)
        #            C = 4 * s_x * s_w (output scale)
        def scales(gm, hist, sfx):
            hmx = small.tile([1, 1], F32, name=f"hmx_{sfx}")
            nc.vector.reduce_max(hmx[:], hist[:, 0 : hl - 1], axis=AX.X)
            amax = small.tile([1, 1], F32, name=f"amax_{sfx}")
            nc.vector.tensor_tensor(amax[:], gm, hmx[:], op=ALU.max)
            ra = small.tile([1, 1], F32, name=f"ra_{sfx}")
            nc.vector.reciprocal(ra[:], amax[:])
            sf = small.tile([1, 1], F32, name=f"sf_{sfx}")
            nc.vector.tensor_scalar_mul(sf[:], ra[:], E4M3_MAX)
            s = small.tile([1, 1], F32, name=f"s_{sfx}")
            nc.vector.reciprocal(s[:], sf[:])
            rh = small.tile([1, 1], F32, name=f"rh_{sfx}")
            nc.vector.tensor_scalar_mul(rh[:], sf[:], 0.5)
            return s, rh

        s_x, rh_x = scales(gmax_x[:], histx, "x")
        s_w, rh_w = scales(gmax_w[:], histw, "w")

        Cs = small.tile([1, 1], F32)
        nc.vector.tensor_tensor(Cs[:], s_x[:], s_w[:], op=ALU.mult)
        nc.vector.tensor_scalar_mul(Cs[:], Cs[:], 4.0)
        rC = small.tile([1, 1], F32)
        nc.vector.reciprocal(rC[:], Cs[:])

        rhx_b = small.tile([P, 1], F32)
        nc.gpsimd.partition_broadcast(rhx_b[:], rh_x[:])
        rhw_b = small.tile([P, 1], F32)
        nc.gpsimd.partition_broadcast(rhw_b[:], rh_w[:])
        C_b = small.tile([P, 1], F32)
        nc.gpsimd.partition_broadcast(C_b[:], Cs[:])

        # bias: fp32 -> bf16 -> fp32, then pre-divide by C, broadcast to 128 parts
        btmp = small.tile([1, f_shard], F32)
        nc.sync.dma_start(btmp[:], b_sh[:])
        bbf = small.tile([1, f_shard], BF16)
        nc.vector.tensor_copy(bbf[:], btmp[:])
        nc.vector.tensor_copy(btmp[:], bbf[:])
        nc.vector.tensor_scalar_mul(btmp[:], btmp[:], rC[:])
        bP = small.tile([P, f_shard], F32)
        nc.gpsimd.partition_broadcast(bP[:], btmp[:])

        # ---- phase 3: quantize + transpose x shard -> qxT (fp8) ----
        # The shard is split into two token-halves, each with its own
        # SBUF-resident tile, DRAM staging block, and AllGather. The two AGs
        # pipeline on the collective engine: foreign-block matmuls can start
        # as soon as AG_A lands (full contraction depth - the split is on
        # tokens, not d), and own-block matmuls only wait for their half's
        # quantization, not the whole shard.
        TH = t_shard // 2
        th_tiles = TH // P
        qxt_dram = dram.tile([2, d, TH], FP8)
        qxt_sb_h = [
            qxs.tile([P, d_tiles, TH], FP8, name=f"qxtsb{h}") for h in range(2)
        ]
        qxt_all_h = [
            dram.tile([n_cores * d, TH], FP8, addr_space=shared, name=f"qxtall{h}")
            for h in range(2)
        ]
        for th in range(2):
            for it in range(th_tiles):
                i = th * th_tiles + it
                for h in range(d_chunks):
                    xt = xs.tile([P, d_half], F32)
                    nc.sync.dma_start(
                        xt[:],
                        x_sh[i * P : (i + 1) * P, h * d_half : (h + 1) * d_half],
                    )
                    for j in range(d_half // P):
                        dj = h * (d_half // P) + j
                        pt = psum.tile([P, NF], F32, tag="ps")
                        nc.tensor.transpose(
                            pt[:, 0:P], xt[:, j * P : (j + 1) * P], ident[:]
                        )
                        nc.vector.tensor_scalar(
                            out=qxt_sb_h[th][:, dj, it * P : (it + 1) * P],
                            in0=pt[:, 0:P],
                            scalar1=rhx_b[:],
                            scalar2=None,
                            op0=ALU.mult,
                        )
            for dj in range(d_tiles):
                nc.sync.dma_start(
                    qxt_dram[th, dj * P : (dj + 1) * P, :], qxt_sb_h[th][:, dj, :]
                )
            nc.gpsimd.collective_compute(
                "AllGather",
                ALU.bypass,
                replica_groups=rg,
                ins=[qxt_dram[th].opt()],
                outs=[qxt_all_h[th][:].opt()],
            )

        # ---- phase 4: quantize w shard (SBUF resident) ----
        # one tile per DoubleRow pair so the matmul k-loop can stream behind
        # the quantization instead of waiting for the whole [d, f] tensor.
        qw_tiles = [
            qwp.tile([P, 2, f_shard], FP8, name=f"qwt{s}")
            for s in range(d_tiles // 2)
        ]
        for j in range(d_tiles):
            wt = ws.tile([P, f_shard], F32)
            nc.sync.dma_start(wt[:], w_sh[j * P : (j + 1) * P, :])
            nc.vector.tensor_scalar(
                out=qw_tiles[j // 2][:, j % 2, :],
                in0=wt[:],
                scalar1=rhw_b[:],
                scalar2=None,
                op0=ALU.mult,
            )

        # ---- phase 5: matmul + epilogue ----
        # out[tok, f] = gelu(C * (sum_d qxT[d, tok] * qw[d, f] + bias/C))
        # Block order is rotated per core: slot 0 is the core's OWN token
        # block, read straight from SBUF (qxt_sb) with no AllGather
        # dependency, so the PE starts while the AG is in flight. Slots 1..7
        # read the AG result at a runtime (partition-id based) offset, and
        # every output DMA row offset is runtime-computed to land the slot at
        # its global token position.
        pid_g = nc.gpsimd.partition_id()
        pid_s = nc.sync.partition_id()

        def mm_chunk(lh, mts, mc0, row0):
            # one chunk: mts m-tiles starting at column mc0 of lh, output
            # rows starting at row0 (RuntimeValue)
            for mt in range(mts):
                pss = [
                    psum.tile([P, NF], F32, tag="ps", name=f"mmps{n}")
                    for n in range(n_tiles)
                ]
                mc = mc0 + mt * P
                for ks in range(d_tiles // 2):
                    for n in range(n_tiles):
                        nc.tensor.matmul(
                            pss[n][:],
                            lhsT=lh[:, 2 * ks : 2 * ks + 2, mc : mc + P],
                            rhs=qw_tiles[ks][:, :, n * NF : (n + 1) * NF],
                            start=(ks == 0),
                            stop=(ks == d_tiles // 2 - 1),
                            perf_mode=DR,
                        )
                row = row0 + mt * P
                for n in range(n_tiles):
                    t1 = stg.tile([P, NF], F32)
                    nc.vector.tensor_tensor(
                        t1[:], pss[n][:], bP[:, n * NF : (n + 1) * NF], op=ALU.add
                    )
                    ot = stg.tile([P, NF], F32)
                    nc.scalar.activation(
                        ot[:], t1[:], act_fn, bias=zbias[:], scale=C_b[:]
                    )
                    nc.gpsimd.dma_start(
                        out_sh[bass.ds(row, P), n * NF : (n + 1) * NF], ot[:]
                    )

        for slot in range(n_cores):
            row_g = ((pid_g + slot) % n_cores) * t_shard
            if slot == 0:
                for th in range(2):
                    mm_chunk(qxt_sb_h[th], TH // P, 0, row_g + th * TH)
            else:
                bi_s = (pid_s + slot) % n_cores
                for th in range(2):
                    src = qxt_all_h[th][bass.ds(bi_s * d, d), :].rearrange(
                        "(s p) m -> p s m", p=P, s=d_tiles
                    )
                    for q in range(TH // MSUP):
                        lh = lhsp.tile([P, d_tiles, MSUP], FP8, name="lh")
                        nc.sync.dma_start(
                            lh[:], src[:, :, q * MSUP : (q + 1) * MSUP]
                        )
                        mm_chunk(
                            lh, MSUP // P, 0, row_g + th * TH + q * MSUP
                        )

    nc.compile()
    return nc


_CACHE = {}


def _get_program(t_shard=T // NCORES, d=D, f_shard=F // NCORES, n_cores=NCORES):
    key = (t_shard, d, f_shard, n_cores)
    if key not in _CACHE:
        _CACHE[key] = build_program(t_shard, d, f_shard, n_cores)
    return _CACHE[key]


def make_in_maps(x, w, bias, in_hist, k_hist, n_cores=NCORES):
    t_shard = x.shape[0] // n_cores
    f_shard = w.shape[1] // n_cores
    in_maps = []
    for r in range(n_cores):
        in_maps.append(
            {
                "x_shard": np.ascontiguousarray(
                    x[r * t_shard : (r + 1) * t_shard], dtype=np.float32
                ),
                "w_shard": np.ascontiguousarray(
                    w[:, r * f_shard : (r + 1) * f_shard], dtype=np.float32
                ),
                "bias_shard": np.ascontiguousarray(
                    bias[r * f_shard : (r + 1) * f_shard], dtype=np.float32
                ).reshape(1, f_shard),
                "in_hist": np.asarray(in_hist, np.float32).reshape(1, HL),
                "k_hist": np.asarray(k_hist, np.float32).reshape(1, HL),
            }
        )
    return in_maps


def _install_ntff_shim():
    """Provide antenv.axon_hooks (absent in this image) so bass_utils can
    NTFF-profile under axon, wiring it to libaxon_pjrt's nrt profile API."""
    import sys as _sys
    import types

    if "antenv.axon_hooks" in _sys.modules:
        return
    mod = types.ModuleType("antenv.axon_hooks")
    _state = {"hook": None}
    mod.set_axon_ntff_profile_hook = lambda h: _state.__setitem__("hook", h)
    mod.get_axon_ntff_profile_hook = lambda: _state["hook"]
    _sys.modules["antenv.axon_hooks"] = mod
    import antenv

    antenv.axon_hooks = mod
    try:
        from trn_agent_boot.trn_boot import _ntff_profile_via_ctypes

        mod.set_axon_ntff_profile_hook(
            _ntff_profile_via_ctypes("/opt/axon/libaxon_pjrt.so")
        )
    except Exception as e:
        print(f"ntff shim: hook unavailable ({e}); tracing will degrade")


def run(inputs_dict, trace=False, **kw):
    """Run on the 8 NeuronCores; returns (full_output, BassKernelResults)."""
    if trace:
        _install_ntff_shim()
    x = np.asarray(inputs_dict["inputs"], np.float32).reshape(T, D)
    w = np.asarray(inputs_dict["kernel"], np.float32)
    bias = np.asarray(inputs_dict["bias"], np.float32)
    nc = _get_program()
    in_maps = make_in_maps(
        x, w, bias, inputs_dict["input_amax_history"], inputs_dict["kernel_amax_history"]
    )
    old_m = nc.m
    nc.m = get_hw_module(nc.m)
    try:
        res = run_bass_kernel_spmd(
            nc, in_maps, core_ids=list(range(NCORES)), trace=trace, **kw
        )
    finally:
        nc.m = old_m
    f_shard = F // NCORES
    out = np.concatenate(
        [res.results[r]["out_shard"] for r in range(NCORES)], axis=1
    )
    return out.reshape(B, S, F).astype(np.float32), res


def kernel(**inputs):
    out, _ = run(inputs, trace=False)
    return out
